# revision 11
# baseline (speedup 1.0000x reference)
"""DualAttention (CAM + PAM) Trainium2 Bass kernel.

Problem (per batch b of 4, C=64 channels, N=8192 positions):
  CAM: A = x@x^T (64x64 gram); att_c = softmax(rowmax(A)-A, axis=0);
       cam = gamma_cam * att_c @ x + x
  PAM: q,k (8,N), v (64,N) via 1x1 convs; att_p = softmax(q^T k, axis=-1)
       pam = gamma_pam * v @ att_p^T + x
  out = cam + pam

Sharding: 8 cores = (batch b in 0..3) x (query-half h in 0..1). Each core
computes the full CAM+PAM output for its 4096 query columns, streaming the
full 8192-wide key/value range (flash-attention style, nothing n^2 ever
touches HBM). Host-side preprocessing only rolls/pads x and re-lays-out the
tiny 1x1-conv weights; all FLOPs run on device.

Key layout trick: scores are computed transposed, S_T[k_chunk, q] so the
exp'd tile feeds the PV matmul directly as the moving operand (no on-chip
transpose of the big attention matrix). The softmax denominator comes for
free from a ones-column appended to v^T (column sums accumulate in PSUM row
64 of the PV output).
"""

import numpy as np

B, C, N = 4, 64, 8192
CQK = C // 8
NCORES = 8

_prog_cache = {}


def _build(Ntot, NH):
    import concourse.bass as bass  # noqa: F401
    import concourse.bacc as bacc
    import concourse.tile as tile
    from concourse import mybir
    from contextlib import ExitStack

    f32 = mybir.dt.float32
    f32r = mybir.dt.float32r
    AF = mybir.ActivationFunctionType
    Alu = mybir.AluOpType
    X = mybir.AxisListType.X

    NCH = Ntot // 128      # 128-wide key chunks
    NT = NH // 512         # query tiles
    KT = Ntot // 512       # 512-wide column tiles of full range

    nc = bacc.Bacc("TRN2", target_bir_lowering=False, debug=False)
    xr_d = nc.dram_tensor("xr", [65, Ntot], f32r, kind="ExternalInput")
    wq_d = nc.dram_tensor("wq", [65, 65], f32r, kind="ExternalInput")
    wk_d = nc.dram_tensor("wk", [65, 65], f32r, kind="ExternalInput")
    wv_d = nc.dram_tensor("wv", [65, 66], f32r, kind="ExternalInput")
    aux_d = nc.dram_tensor("aux", [64, 66], f32, kind="ExternalInput")
    id_d = nc.dram_tensor("ident", [65, 65], f32, kind="ExternalInput")
    y_d = nc.dram_tensor("y", [64, NH], f32, kind="ExternalOutput")

    with tile.TileContext(nc) as tc, ExitStack() as ctx:
        sb = ctx.enter_context(tc.tile_pool(name="sb", bufs=1))
        xr_sb = sb.tile([65, Ntot], f32r)
        wq_sb = sb.tile([65, 65], f32r)
        wk_sb = sb.tile([65, 65], f32r)
        wv_sb = sb.tile([65, 66], f32r)
        aux_sb = sb.tile([64, 66], f32)
        id_sb = sb.tile([65, 65], f32)
        q_sb = sb.tile([65, NH], f32r)
        k_sb = sb.tile([65, Ntot], f32r)
        vT_sb = sb.tile([128, NCH, 65], f32r)
        xT_sb = sb.tile([128, NCH, 65], f32)
        cam_sb = sb.tile([64, NH], f32)
        # ones-row matrix (row 0 = 1, rest 0) and a persistent zero-padded
        # reciprocal row: used to broadcast 1/s across partitions via the PE.
        ones_sb = sb.tile([65, 65], f32)
        nc.vector.memset(ones_sb[:, :], 0.0)
        nc.vector.memset(ones_sb[0:1, :], 1.0)
        rs_sb = sb.tile([65, 512], f32)
        nc.vector.memset(rs_sb[:, :], 0.0)

        nc.sync.dma_start(xr_sb[:, :], xr_d[:, :])
        nc.sync.dma_start(wq_sb[:, :], wq_d[:, :])
        nc.sync.dma_start(wk_sb[:, :], wk_d[:, :])
        nc.sync.dma_start(wv_sb[:, :], wv_d[:, :])
        nc.sync.dma_start(aux_sb[:, :], aux_d[:, :])
        nc.sync.dma_start(id_sb[:, :], id_d[:, :])

        with tc.tile_pool(name="psA", space="PSUM", bufs=2) as psA:
            # --- q, k production: rows 0-7 live, rows 8-64 exact zeros (pads
            # the contract dim so every matmul runs in (128,128) array mode).
            for t in range(NT):
                qp = psA.tile([65, 512], f32, tag="pp")
                nc.tensor.matmul(qp[:, :], wq_sb[:, :], xr_sb[:, t * 512:(t + 1) * 512])
                nc.scalar.copy(q_sb[:, t * 512:(t + 1) * 512], qp[:, :])
            for t in range(KT):
                kp = psA.tile([65, 512], f32, tag="pp")
                nc.tensor.matmul(kp[:, :], wk_sb[:, :], xr_sb[:, t * 512:(t + 1) * 512])
                nc.scalar.copy(k_sb[:, t * 512:(t + 1) * 512], kp[:, :])

            # --- v^T (n-major) with a ones column at 64 for softmax sums.
            for g in range(NCH // 4):
                vp = psA.tile([128, 4, 128], f32, tag="ppv")
                for j in range(4):
                    ch = 4 * g + j
                    nc.tensor.matmul(
                        vp[:, j, 0:66], xr_sb[:, ch * 128:(ch + 1) * 128], wv_sb[:, :]
                    )
                nc.vector.tensor_copy(vT_sb[:, 4 * g:4 * g + 4, :], vp[:, :, 0:65])

            # --- x^T (exact fp32, PE transpose) for the CAM gram matrix.
            for g in range(NCH // 4):
                xp = psA.tile([128, 4, 128], f32, tag="ppx")
                for j in range(4):
                    ch = 4 * g + j
                    nc.tensor.transpose(
                        xp[:, j, 0:65],
                        xr_sb[:, ch * 128:(ch + 1) * 128].bitcast(f32),
                        id_sb[:, :],
                    )
                nc.vector.tensor_copy(xT_sb[:, 4 * g:4 * g + 4, :], xp[:, :, 0:65])

            # --- A = x@x^T (fp32 accumulate over chunks)
            A_ps = psA.tile([65, 65], f32, tag="ppa")
            for i in range(NCH):
                nc.tensor.matmul(
                    A_ps[:, :], xT_sb[:, i, :], xT_sb[:, i, :],
                    start=(i == 0), stop=(i == NCH - 1),
                )

            # --- CAM softmax chain (64x64, cheap)
            m_sb = sb.tile([64, 1], f32)
            nc.vector.tensor_reduce(m_sb[:, :], A_ps[0:64, 0:64], axis=X, op=Alu.max)
            bm_sb = sb.tile([64, 64], f32)
            nc.vector.tensor_scalar(
                bm_sb[:, :], A_ps[0:64, 0:64], m_sb[:, :], None, op0=Alu.subtract
            )
            bt_ps = psA.tile([64, 64], f32, tag="ppa")
            nc.tensor.transpose(bt_ps[:, :], bm_sb[:, :], id_sb[0:64, 0:64])
            mn_sb = sb.tile([64, 1], f32)
            nc.vector.tensor_reduce(mn_sb[:, :], bt_ps[:, :], axis=X, op=Alu.min)
            expe_sb = sb.tile([64, 64], f32)
            sc_sb = sb.tile([64, 1], f32)
            nc.scalar.activation(
                expe_sb[:, :], bt_ps[:, :], AF.Exp,
                scale=-1.0, bias=mn_sb[:, :], accum_out=sc_sb[:, :],
            )
            rc_sb = sb.tile([64, 1], f32)
            nc.vector.reciprocal(rc_sb[:, :], sc_sb[:, :])
            att_sb = sb.tile([64, 64], f32)
            nc.vector.tensor_scalar(
                att_sb[:, :], expe_sb[:, :], rc_sb[:, :], aux_sb[:, 64:65],
                op0=Alu.mult, op1=Alu.mult,
            )
            # att2 = gamma_cam*att^T + 2I, padded to (65,65) with zeros
            att2_sb = sb.tile([64, 64], f32r)
            nc.vector.tensor_add(att2_sb[:, :], att_sb[:, :], aux_sb[:, 0:64])

            # cam2 = att2^T @ x  (= gamma_cam*cam + 2x), for this core's span
            for t in range(NT):
                cp = psA.tile([65, 512], f32, tag="pp")
                nc.tensor.matmul(
                    cp[0:64, :], att2_sb[:, :], xr_sb[0:64, t * 512:(t + 1) * 512]
                )
                nc.vector.tensor_copy(cam_sb[:, t * 512:(t + 1) * 512], cp[0:64, :])

        # --- PAM flash-attention loop
        with (
            tc.tile_pool(name="psB", space="PSUM", bufs=3) as psB,
            tc.tile_pool(name="pps", bufs=3) as pps,
            tc.tile_pool(name="tl", bufs=2) as tl,
        ):
            for t in range(NT):
                pv = psB.tile([65, 512], f32, tag="pv", bufs=1)
                qs = q_sb[:, t * 512:(t + 1) * 512]
                nhg = NCH // 2
                for hg in range(nhg):
                    st = psB.tile([128, 2, 512], f32, tag="st")
                    for j in range(2):
                        ch = 2 * hg + j
                        nc.tensor.matmul(
                            st[:, j, :], k_sb[:, ch * 128:(ch + 1) * 128], qs
                        )
                    pt = pps.tile([128, 2, 512], f32r, tag="p")
                    nc.scalar.activation(pt[:, :, :], st[:, :, :], AF.Exp)
                    for j in range(2):
                        ch = 2 * hg + j
                        nc.tensor.matmul(
                            pv[:, :], vT_sb[:, ch, :], pt[:, j, :],
                            start=(hg == 0 and j == 0),
                            stop=(hg == nhg - 1 and j == 1),
                        )
                rs = tl.tile([1, 512], f32, tag="rs")
                nc.vector.reciprocal(rs[:, :], pv[64:65, :])
                nc.vector.tensor_scalar(
                    rs_sb[0:1, :], rs[:, :], aux_sb[0:1, 65:66], None,
                    op0=Alu.mult,
                )
                bc_ps = psB.tile([65, 512], f32, tag="bc", bufs=1)
                nc.tensor.matmul(bc_ps[:, :], ones_sb[:, :], rs_sb[:, :])
                bc_sb = tl.tile([64, 512], f32, tag="bc")
                nc.vector.tensor_copy(bc_sb[:, :], bc_ps[0:64, :])
                pam_sb = tl.tile([64, 512], f32, tag="pam")
                nc.vector.tensor_mul(pam_sb[:, :], pv[0:64, :], bc_sb[:, :])
                out_sb = tl.tile([64, 512], f32, tag="out")
                nc.vector.tensor_add(
                    out_sb[:, :], pam_sb[:, :], cam_sb[:, t * 512:(t + 1) * 512]
                )
                nc.sync.dma_start(y_d[:, t * 512:(t + 1) * 512], out_sb[:, :])
    nc.compile()
    return nc


def _get_nc(Ntot, NH):
    key = (Ntot, NH)
    if key not in _prog_cache:
        _prog_cache[key] = _build(Ntot, NH)
    return _prog_cache[key]


def _core_inputs(xb, w1, b1, w2, b2, w3, b3, gcam, gpam, half, Ntot, NH):
    xroll = np.roll(xb, -half * NH, axis=1)
    xr = np.concatenate([xroll, np.ones((1, Ntot), np.float32)], axis=0)
    wq = np.zeros((65, 65), np.float32)
    wq[0:64, 0:CQK] = w1.T
    wq[64, 0:CQK] = b1
    wk = np.zeros((65, 65), np.float32)
    wk[0:64, 0:CQK] = w2.T
    wk[64, 0:CQK] = b2
    wv = np.zeros((65, 66), np.float32)
    wv[0:64, 0:64] = w3.T
    wv[64, 0:64] = b3
    wv[64, 64] = 1.0
    aux = np.zeros((64, 66), np.float32)
    aux[:, 0:64] = 2.0 * np.eye(64, dtype=np.float32)
    aux[:, 64] = gcam
    aux[:, 65] = gpam
    ident = np.eye(65, dtype=np.float32)
    return {
        "xr": np.ascontiguousarray(xr),
        "wq": wq, "wk": wk, "wv": wv, "aux": aux, "ident": ident,
    }


def kernel(x, w1, b1, w2, b2, w3, b3, gamma_cam, gamma_pam):
    from concourse.bass_utils import run_bass_kernel_spmd

    x = np.asarray(x, dtype=np.float32)
    w1 = np.asarray(w1, dtype=np.float32)
    b1 = np.asarray(b1, dtype=np.float32)
    w2 = np.asarray(w2, dtype=np.float32)
    b2 = np.asarray(b2, dtype=np.float32)
    w3 = np.asarray(w3, dtype=np.float32)
    b3 = np.asarray(b3, dtype=np.float32)
    gcam = float(np.asarray(gamma_cam).reshape(-1)[0])
    gpam = float(np.asarray(gamma_pam).reshape(-1)[0])

    NH = N // 2
    nc = _get_nc(N, NH)
    in_maps = []
    for core in range(NCORES):
        b, half = core // 2, core % 2
        in_maps.append(
            _core_inputs(x[b], w1, b1, w2, b2, w3, b3, gcam, gpam, half, N, NH)
        )
    res = run_bass_kernel_spmd(nc, in_maps, core_ids=list(range(NCORES)))
    y = np.empty((B, C, N), dtype=np.float32)
    for core in range(NCORES):
        b, half = core // 2, core % 2
        y[b, :, half * NH:(half + 1) * NH] = res.results[core]["y"]
    return y
